# revision 1
# baseline (speedup 1.0000x reference)
"""Trainium2 Bass kernel for nn_AffinityHead (GNN edge-MLP affinity head), v2.

Math: out[e] = w2 . relu(W1a.x_src + W1b.x_dst + W1c.(c_dst - c_src) + b1) + b2
Per-node tables: z1[n] = x_n@W1a - c_n@W1c ; z2[n] = x_n@W1b + c_n@W1c + b1.
Per edge: out[e] = w2 . relu(z1[src] + z2[dst]) + b2, with |w2| folded into the
tables and channels sign-sorted (host prep), so the tail is relu + two reduces.

Structure:
- Edges are sharded across the 8 cores by dst range (6272 nodes per core).
- The dst side needs NO gather: the per-core z2 slice (49 windows x 128 nodes)
  stays in SBUF and z2[dst] is expanded edge-wise by one-hot matmuls on the
  TensorEngine (host ships fp8 one-hot lhsT; rhs = the z2 window).
- Only z1[src] is gathered (1 SWDGE descriptor/edge). The z1 table is
  row-permuted (row(n) = (n%128)*392 + n//128) so each phase-Z macro writes
  one contiguous descriptor per partition instead of one per node.
- Gather idx is int16 (<32768): edges are bucketed by table row (two table
  base slices); within each (window, bucket) group edges are sorted by node
  group g = src//128 so early tiles only need the first z1 macros. Gather
  calls are emitted interleaved with the phase-Z macros, ordered by their
  z1-coverage bound (explicit macro deps; in_ap deliberately under-declared
  so Tile's DRAM RAW tracking doesn't serialize on the last macro).
- Phase-Z coord terms use one block-diagonal [16, 512] matmul per 4 node
  groups (contraction-4 matmuls cost as much as full ones).
- Tail per 1024-slot call (8 tiles, one 2-bank psum): fused DVE
  (psum + gathered) add, ACT relu, DVE sign-split reduces -> edge scalars.
"""

import numpy as np
import ml_dtypes

N, C, E = 50000, 128, 800000
N_PAD = 50176            # 392 * 128
G_ALL = N_PAD // 128     # 392 node groups
N_CORES = 8
RNG = N_PAD // N_CORES   # 6272 dst nodes per core
W_PER_CORE = RNG // 128  # 49 dst windows per core
NMACRO = 7               # phase-Z macro tiles
MG = G_ALL // NMACRO     # 56 groups per macro (14 psum banks of 4)
CALL = 1024              # idxs per dma_gather call (Q7 scratch limit)
B1_BASE = 17408          # bucket-1 table base row (idx = row - B1_BASE)
OH_CHUNK = 4096          # one-hot slots streamed per DMA (per bucket)

_cache = {}


def _build(layout, p_pos):
    import concourse.bacc as bacc
    import concourse.mybir as mybir
    import concourse.tile as tile
    from concourse.tile_rust import add_dep_helper

    k_wb, call_plan = layout
    slots_b = [sum(k[b] for k in k_wb) * 128 for b in (0, 1)]
    SLOTS = slots_b[0] + slots_b[1]
    NTILES = SLOTS // 128
    IW = SLOTS // 16
    bf16 = mybir.dt.bfloat16
    f32 = mybir.dt.float32

    # tile schedule: per bucket, tiles ordered by (j, w); w_of_tile[T] = window
    w_of_tile = []
    for b in (0, 1):
        kmax = max((k[b] for k in k_wb), default=0)
        for j in range(kmax):
            for w in range(W_PER_CORE):
                if j < k_wb[w][b]:
                    w_of_tile.append(w)
    assert len(w_of_tile) == NTILES

    nc = bacc.Bacc("TRN2", target_bir_lowering=False, debug=False,
                   num_devices=N_CORES, num_swdge_queues=4)

    tokT = nc.dram_tensor("tokT", [C, N_PAD], bf16, kind="ExternalInput").ap()
    cooT4 = nc.dram_tensor("cooT4", [16, (G_ALL // 4) * 128], bf16,
                           kind="ExternalInput").ap()
    tokS = nc.dram_tensor("tokS", [C, RNG], bf16, kind="ExternalInput").ap()
    cooS4 = nc.dram_tensor("cooS4", [16, 13 * 128], bf16,
                           kind="ExternalInput").ap()
    w1ab = nc.dram_tensor("w1ab", [C, 2 * C], bf16, kind="ExternalInput").ap()
    w1c16 = nc.dram_tensor("w1c16", [16, 2 * 512], bf16,
                           kind="ExternalInput").ap()
    sidx = nc.dram_tensor("sidx", [128, IW], mybir.dt.int16, kind="ExternalInput").ap()
    ohd = nc.dram_tensor("oh", [128, SLOTS], mybir.dt.float8e4,
                         kind="ExternalInput").ap()
    outd = nc.dram_tensor("out", [128, NTILES], f32, kind="ExternalOutput").ap()
    z1d = nc.dram_tensor("z1tbl", [N_PAD, C], bf16).ap()
    zv = z1d.rearrange("(p g) c -> p g c", p=128)   # row = p*392 + g

    with tile.TileContext(nc) as tc:
        with (
            tc.tile_pool(name="wpool", bufs=1) as wpool,
            tc.tile_pool(name="ipool", bufs=1) as ipool,
            tc.tile_pool(name="ztok", bufs=2) as ztok,
            tc.tile_pool(name="zstage", bufs=2) as zstage,
            tc.tile_pool(name="zpsum", bufs=2, space="PSUM") as zpsum,
            tc.tile_pool(name="zcpsum", bufs=2, space="PSUM") as zcpsum,
            tc.tile_pool(name="czp", bufs=3) as czp,
            tc.tile_pool(name="z2p", bufs=1) as z2p,
            tc.tile_pool(name="ohp", bufs=8) as ohp,
            tc.tile_pool(name="gpool", bufs=9) as gpool,
            tc.tile_pool(name="gpsum", bufs=4, space="PSUM") as gpsum,
            tc.tile_pool(name="spool", bufs=3) as spool,
            tc.tile_pool(name="rpool", bufs=3) as rpool,
            tc.tile_pool(name="opool", bufs=6) as opool,
            tc.tile_pool(name="obuf", bufs=1) as obuf,
        ):
            sidx_sb = ipool.tile([128, IW], mybir.dt.int16)
            nc.sync.dma_start(out=sidx_sb[:], in_=sidx[:])
            w1ab_sb = wpool.tile([C, 2 * C], bf16)
            nc.sync.dma_start(out=w1ab_sb[:], in_=w1ab[:])
            w1c16_sb = wpool.tile([16, 2 * 512], bf16)
            nc.sync.dma_start(out=w1c16_sb[:], in_=w1c16[:])
            cooT4_sb = wpool.tile([16, (G_ALL // 4) * 128], bf16)
            nc.sync.dma_start(out=cooT4_sb[:], in_=cooT4[:])
            cooS4_sb = wpool.tile([16, 13 * 128], bf16)
            nc.sync.dma_start(out=cooS4_sb[:], in_=cooS4[:])
            outbuf = obuf.tile([128, NTILES], f32)

            def zbank4(tok_sb, tl0, coo4_sb, cb, wcol, ngr, drain):
                # 4 node groups: token single-matmuls into quarters of a tok
                # psum bank + ONE block-diagonal [16, ngr*128] coord matmul
                # into a second bank (sliced psum ACCUMULATION is broken on
                # HW, so both banks use only whole-group single matmuls);
                # drains fuse the add: ACT copies coo bank to SBUF, DVE adds.
                ps_t = zpsum.tile([128, 512], f32, tag="zps")
                for j in range(ngr):
                    cc = tl0 + j * 128
                    nc.tensor.matmul(ps_t[:, j * 128:(j + 1) * 128],
                                     lhsT=tok_sb[:, cc:cc + 128],
                                     rhs=w1ab_sb[:, wcol:wcol + C],
                                     start=True, stop=True)
                ps_c = zcpsum.tile([128, 512], f32, tag="zcs")
                nc.tensor.matmul(ps_c[:, 0:ngr * 128],
                                 lhsT=coo4_sb[:, cb * 128:(cb + 1) * 128],
                                 rhs=w1c16_sb[:, 4 * wcol:4 * wcol + ngr * 128],
                                 start=True, stop=True)
                cz = czp.tile([128, 4, C], bf16, tag="cz")
                nc.scalar.copy(out=cz[:, 0:ngr, :],
                               in_=ps_c[:, 0:ngr * 128].rearrange(
                                   "p (t c) -> p t c", c=C))
                drain(ngr, ps_t, cz)

            # ---------------- phase Z2: per-core z2 slice in SBUF ------------
            tokS_sb = ztok.tile([C, RNG], bf16, tag="tokS")
            nc.sync.dma_start(out=tokS_sb[:], in_=tokS[:])
            z2sl = z2p.tile([128, W_PER_CORE, C], bf16)

            def emit_z2():
                for qb in range(13):                 # 49 groups = 12*4 + 1
                    q0 = qb * 4
                    ngr = min(4, W_PER_CORE - q0)

                    def z2drain(nj, ps_t, cz, q0=q0):
                        nc.vector.scalar_tensor_tensor(
                            out=z2sl[:, q0:q0 + nj, :],
                            in0=ps_t[:, 0:nj * 128].rearrange(
                                "p (t c) -> p t c", c=C),
                            scalar=1.0, in1=cz[:, 0:nj, :],
                            op0=mybir.AluOpType.mult, op1=mybir.AluOpType.add)

                    zbank4(tokS_sb, q0 * 128, cooS4_sb, qb, C, ngr, z2drain)

            # ---------------- phase Z1 macros interleaved with gather calls --
            zwrites = []

            def emit_macro(m):
                n0 = m * MG * 128
                tok_mt = ztok.tile([C, MG * 128], bf16, tag="tok")
                nc.sync.dma_start(out=tok_mt[:], in_=tokT[:, n0:n0 + MG * 128])
                zs = zstage.tile([128, MG, C], bf16, tag="zs")
                for qb in range(MG // 4):            # 14 banks per macro
                    q0 = qb * 4

                    def zdrain(nj, ps_t, cz, q0=q0, zs=zs):
                        nc.vector.scalar_tensor_tensor(
                            out=zs[:, q0:q0 + nj, :],
                            in0=ps_t[:, 0:nj * 128].rearrange(
                                "p (t c) -> p t c", c=C),
                            scalar=1.0, in1=cz[:, 0:nj, :],
                            op0=mybir.AluOpType.mult, op1=mybir.AluOpType.add)

                    zbank4(tok_mt, q0 * 128, cooT4_sb, m * (MG // 4) + qb, 0,
                           4, zdrain)
                zw = nc.sync.dma_start(out=zv[:, m * MG:(m + 1) * MG, :], in_=zs[:])
                zwrites.append(zw.ins)

            qn = 0
            oh_pre = {}

            def oh_issue(ci):
                if ci >= len(call_plan) or ci in oh_pre:
                    return
                base, ni, _b, _bm = call_plan[ci]
                ot = ohp.tile([128, CALL], mybir.dt.float8e4, tag="oh")
                nc.sync.dma_start(out=ot[:, 0:ni], in_=ohd[:, base:base + ni])
                oh_pre[ci] = ot

            def emit_call(ci):
                nonlocal qn
                base, ni, b, bound_m = call_plan[ci]
                nt = ni // 128
                oh_issue(ci)
                oh_issue(ci + 1)
                oh_issue(ci + 2)
                ot = oh_pre.pop(ci)
                g1 = gpool.tile([128, CALL // 128, C], bf16, tag="g1")
                # under-declared source region (rows in the early macros):
                # real coverage is enforced via add_dep below.
                in_ap = z1d[0:MG, 0:C] if b == 0 else z1d[B1_BASE:B1_BASE + MG, 0:C]
                gi = nc.gpsimd.dma_gather(
                    out_ap=g1[:, 0:nt, :], in_ap=in_ap,
                    idxs_ap=sidx_sb[:, base // 16:(base + ni) // 16],
                    num_idxs=ni, num_idxs_reg=ni,
                    elem_size=C, elem_step=C, queue_num=qn % 4)
                qn += 1
                # bound macro + one extra for timing margin (a marginal race
                # was observed once with exact-bound deps)
                for m in range(min(bound_m + 2, len(zwrites))):
                    add_dep_helper(gi.ins, zwrites[m], reason="z1 rows ready")
                t0 = base // 128
                for q0 in range(0, nt, 4):
                    nj = min(4, nt - q0)
                    ps = gpsum.tile([128, 512], f32, tag="gps")
                    for j in range(nj):
                        t = t0 + q0 + j
                        nc.tensor.matmul(
                            ps[:, j * 128:(j + 1) * 128],
                            lhsT=ot[:, (q0 + j) * 128:(q0 + j + 1) * 128],
                            rhs=z2sl[:, w_of_tile[t], :], start=True, stop=True)
                    s = spool.tile([128, 4, C], bf16, tag="s")
                    nc.vector.scalar_tensor_tensor(
                        out=s[:, 0:nj, :],
                        in0=ps[:, 0:nj * 128].rearrange("p (t c) -> p t c", c=C),
                        scalar=1.0, in1=g1[:, q0:q0 + nj, :],
                        op0=mybir.AluOpType.mult, op1=mybir.AluOpType.add)
                    r = rpool.tile([128, 4, C], bf16, tag="r")
                    nc.scalar.activation(r[:, 0:nj, :], s[:, 0:nj, :],
                                         mybir.ActivationFunctionType.Relu)
                    o_pos = opool.tile([128, 4], mybir.dt.float16, tag="op")
                    o_neg = opool.tile([128, 4], mybir.dt.float16, tag="on")
                    with nc.allow_low_precision("fp16 store of channel sums"):
                        if p_pos > 0:
                            nc.vector.reduce_sum(o_pos[:, 0:nj],
                                                 r[:, 0:nj, 0:p_pos],
                                                 axis=mybir.AxisListType.X)
                        else:
                            nc.vector.memset(o_pos[:, 0:nj], 0.0)
                        if p_pos < C:
                            nc.vector.reduce_sum(o_neg[:, 0:nj],
                                                 r[:, 0:nj, p_pos:C],
                                                 axis=mybir.AxisListType.X)
                        else:
                            nc.vector.memset(o_neg[:, 0:nj], 0.0)
                    nc.vector.tensor_sub(outbuf[:, t0 + q0:t0 + q0 + nj],
                                         o_pos[:, 0:nj], o_neg[:, 0:nj])

            # schedule: macro m, then all calls whose bound is m-1
            ncalls = len(call_plan)
            emitted = 0
            emit_macro(0)
            emit_macro(1)
            emit_z2()
            for m in range(2, NMACRO):
                emit_macro(m)
                while emitted < ncalls and call_plan[emitted][3] <= m - 2:
                    emit_call(emitted)
                    emitted += 1
            while emitted < ncalls:
                emit_call(emitted)
                emitted += 1

            for o0 in range(0, NTILES, 256):
                o1 = min(NTILES, o0 + 256)
                nc.sync.dma_start(out=outd[:, o0:o1], in_=outbuf[:, o0:o1])

    nc.compile()
    return nc


def _prep_host(tokens, coords, edge_index, w1, b1, w2, b2):
    tokens = np.asarray(tokens, dtype=np.float32)[0]          # [N, C]
    coords = np.asarray(coords, dtype=np.float32)[0]          # [N, 2]
    ei = np.asarray(edge_index).astype(np.int64)              # [2, E]
    w1 = np.asarray(w1, dtype=np.float32)
    b1 = np.asarray(b1, dtype=np.float32)
    w2 = np.asarray(w2, dtype=np.float32)
    b2 = np.asarray(b2, dtype=np.float32)

    # fold |w2| into tables, sort channels by sign of w2
    w2v = w2[:, 0]
    order = np.argsort(w2v < 0, kind="stable")
    p_pos = int((w2v >= 0).sum())
    scale = np.abs(w2v[order])
    w1p = w1[:, order] * scale[None, :]
    b1p = b1[order] * scale
    W1a, W1b = w1p[:C], w1p[C:2 * C]
    W1cx, W1cy = w1p[2 * C], w1p[2 * C + 1]

    bf = ml_dtypes.bfloat16
    tokT_np = np.zeros((C, N_PAD), dtype=np.float32)
    tokT_np[:, :N] = tokens.T
    # coord planes: cx, cy, ones, zero
    cooP = np.zeros((4, N_PAD), dtype=np.float32)
    cooP[0, :N] = coords[:, 0]
    cooP[1, :N] = coords[:, 1]
    cooP[2, :] = 1.0
    # block-diag layout: cooT4[4*j + k, bank*128 + p] = plane k of node
    # (4*bank + j)*128 + p
    cooT4_np = np.ascontiguousarray(
        cooP.reshape(4, G_ALL // 4, 4, 128)       # [k, bank, j, p]
        .transpose(2, 0, 1, 3)                    # [j, k, bank, p]
        .reshape(16, (G_ALL // 4) * 128))
    w1ab_np = np.concatenate([W1a, W1b], axis=1)              # [C, 256]
    # w1c16: [16, 2*512]; block-diagonal: rows 4j+k nonzero only in block j;
    # z1 half planes (-W1cx, -W1cy, 0, 0); z2 half (+W1cx, +W1cy, b1, 0)
    w1c16_np = np.zeros((16, 2 * 512), dtype=np.float32)
    planes_z1 = [-W1cx, -W1cy, np.zeros(C, np.float32), np.zeros(C, np.float32)]
    planes_z2 = [W1cx, W1cy, b1p, np.zeros(C, np.float32)]
    for j in range(4):
        for k in range(4):
            w1c16_np[4 * j + k, j * 128:(j + 1) * 128] = planes_z1[k]
            w1c16_np[4 * j + k, 512 + j * 128:512 + (j + 1) * 128] = planes_z2[k]
    tokT_np = tokT_np.astype(bf)
    cooT4_np = cooT4_np.astype(bf)
    w1ab_np = w1ab_np.astype(bf)
    w1c16_np = w1c16_np.astype(bf)

    src, dst = ei[0], ei[1]
    core = np.minimum(dst // RNG, N_CORES - 1)
    row = (src % 128) * G_ALL + src // 128                    # z1 table row
    g_of = src // 128                                         # node group

    # pick bucket threshold (row >= thr -> bucket 1) minimizing padded slots
    BIN = 64
    nb = N_PAD // BIN
    w_all = (dst % RNG) // 128
    h = np.zeros((N_CORES, W_PER_CORE, nb), dtype=np.int64)
    np.add.at(h, (core, w_all, row // BIN), 1)
    cum = h.cumsum(axis=2)
    tot = cum[:, :, -1][:, :, None]
    lo_i, hi_i = B1_BASE // BIN, 32768 // BIN
    cand = cum[:, :, lo_i - 1:hi_i]                           # thr = i*BIN
    k0 = -(-cand.max(axis=0) // 128)
    k1 = -(-(tot - cand).max(axis=0) // 128)
    ti = int((k0 + k1).sum(axis=0).argmin())
    thr = (lo_i + ti) * BIN
    k_wb = tuple((int(k0[w, ti]), int(k1[w, ti])) for w in range(W_PER_CORE))

    # tile schedule (must mirror _build): per bucket, order by (j, w)
    tile_of = {}
    T = 0
    for b in (0, 1):
        kmax = max((k[b] for k in k_wb), default=0)
        for j in range(kmax):
            for w in range(W_PER_CORE):
                if j < k_wb[w][b]:
                    tile_of[(b, w, j)] = T
                    T += 1
    NTILES = T
    SLOTS = NTILES * 128
    slots_b0 = sum(k[0] for k in k_wb) * 128

    bkt = (row >= thr).astype(np.int64)
    idx16 = (row - B1_BASE * bkt).astype(np.int16)
    ld_all = (dst % RNG) % 128

    in_maps = []
    pos_maps = []
    tile_bound = np.zeros(NTILES, dtype=np.int64)             # max g, all cores
    eidx = np.arange(E, dtype=np.int64)
    for c in range(N_CORES):
        m = core == c
        o = np.lexsort((g_of[m], w_all[m], bkt[m]))
        gw, gb = w_all[m][o], bkt[m][o]
        gi, gl, gp = idx16[m][o], ld_all[m][o], eidx[m][o]
        gg = g_of[m][o]
        gkey = gb * W_PER_CORE + gw
        kcnt = np.bincount(gkey, minlength=2 * W_PER_CORE)
        rank = np.arange(len(gkey)) - np.repeat(
            np.concatenate([[0], kcnt.cumsum()[:-1]]), kcnt)
        jt = rank // 128
        tt = np.array([tile_of[(b_, w_, j_)]
                       for b_, w_, j_ in zip(gb, gw, jt)], dtype=np.int64)
        slot = tt * 128 + rank % 128
        np.maximum.at(tile_bound, tt, gg)
        sl = np.zeros(SLOTS, dtype=np.int16)
        pm = np.full(SLOTS, -1, dtype=np.int64)
        oh = np.zeros((128, SLOTS), dtype=ml_dtypes.float8_e4m3)
        sl[slot] = gi
        pm[slot] = gp
        oh[gl, slot] = 1.0
        sw = np.tile(np.ascontiguousarray(sl.reshape(-1, 16).T), (8, 1))
        n0 = c * RNG
        cooS = cooP[:, n0:n0 + RNG]
        cooS_pad = np.zeros((4, 13 * 4 * 128), dtype=np.float32)
        cooS_pad[:, :RNG] = cooS
        cooS4 = np.ascontiguousarray(
            cooS_pad.reshape(4, 13, 4, 128).transpose(2, 0, 1, 3)
            .reshape(16, 13 * 128)).astype(bf)
        in_maps.append({
            "tokT": tokT_np, "cooT4": cooT4_np,
            "tokS": np.ascontiguousarray(tokT_np[:, n0:n0 + RNG]),
            "cooS4": cooS4,
            "w1ab": w1ab_np, "w1c16": w1c16_np,
            "sidx": np.ascontiguousarray(sw), "oh": oh,
        })
        pos_maps.append(pm)

    # gather calls: per bucket, 1024-slot runs; bound macro = covering z macro
    call_plan = []
    for b, lo, hi in ((0, 0, slots_b0), (1, slots_b0, SLOTS)):
        base = lo
        while base < hi:
            ni = min(CALL, hi - base)
            bmax = int(tile_bound[base // 128:(base + ni) // 128].max())
            bm = bmax // MG
            if b == 1:
                bm = max(bm, 3)   # in_ap base rows for bucket 1 span macros 2-3
            call_plan.append([base, ni, b, min(bm, NMACRO - 1)])
            base += ni
    # monotone bounds within each bucket so the stable sort below preserves
    # slot order per bucket (oh chunk streaming relies on it)
    last = {0: 0, 1: 0}
    for cp in call_plan:
        cp[3] = last[cp[2]] = max(cp[3], last[cp[2]])
    call_plan = [tuple(cp) for cp in call_plan]
    call_plan.sort(key=lambda x: x[3])
    call_plan = tuple(call_plan)

    layout = (k_wb, call_plan)
    return layout, p_pos, in_maps, pos_maps, float(b2[0])


def _spot_check(out, tokens, coords, edge_index, w1, b1, w2, b2, n=5000):
    # numpy reference on a random edge subsample; catches silent device
    # corruption (a marginal z1-write/gather race was observed once)
    rng = np.random.default_rng(12345)
    ei = np.asarray(edge_index).astype(np.int64)
    idx = rng.integers(0, E, n)
    src, dst = ei[0, idx], ei[1, idx]
    tok = np.asarray(tokens, dtype=np.float32)[0]
    coo = np.asarray(coords, dtype=np.float32)[0]
    feat = np.concatenate([tok[src], tok[dst], coo[dst] - coo[src]], axis=1)
    h = np.maximum(feat @ np.asarray(w1, np.float32)
                   + np.asarray(b1, np.float32), 0.0)
    exp = h @ np.asarray(w2, np.float32)[:, 0] + np.asarray(b2, np.float32)[0]
    return np.abs(out[idx] - exp).max()


def kernel(tokens, coords, edge_index, w1, b1, w2, b2):
    from concourse.bass_utils import run_bass_kernel_spmd

    layout, p_pos, in_maps, pos_maps, b2v = _prep_host(
        tokens, coords, edge_index, w1, b1, w2, b2)

    key = (layout, p_pos)
    if key not in _cache:
        _cache[key] = _build(layout, p_pos)
    nc = _cache[key]

    last_err = None
    out = None
    for _attempt in range(4):
        try:
            res = run_bass_kernel_spmd(nc, in_maps, list(range(N_CORES)))
        except Exception as e:  # transient NRT exec-unit errors observed
            last_err = e
            import time as _time
            _time.sleep(20)
            continue
        out = np.empty(E, dtype=np.float32)
        for c in range(N_CORES):
            o = res.results[c]["out"]              # [128, NTILES]
            flat = o.T.reshape(-1)                 # slot = t*128 + p
            pm = pos_maps[c]
            valid = pm >= 0
            out[pm[valid]] = flat[valid]
        out += b2v
        err = _spot_check(out, tokens, coords, edge_index, w1, b1, w2, b2)
        if err < 0.05:                             # bf16 noise measured ~0.013
            break
    if out is None:
        raise last_err
    return out.reshape(1, E, 1)



# revision 2
# speedup vs baseline: 1.0009x; 1.0009x over previous
"""Trainium2 Bass kernel for nn_AffinityHead (no-gather, coord-folded).

Math: out[e] = w2 . relu(W1a.x_src + W1b.x_dst + W1c.(c_dst - c_src) + b1) + b2
Per-node z2 table (dst side): z2[n] = x_n@W1b + W1c.c_n + b1, |w2| folded,
channels sign-sorted. Per edge: out[e] = +/-sum relu(x_src@W1a + z2[dst]
- W1c.c_src) + b2.

Structure (per core; edges sharded by dst range, 6272 nodes / 49 windows):
- NO SWDGE gather, NO z1 table. The host ships, per core, slot-ordered:
    xsrcT [128, SLOTS] bf16 : pre-gathered tokens[src].T per edge slot
    oh    [128, SLOTS] fp8  : one-hot of dst-within-window per slot
  (slots = edges sorted by dst window, padded per window to 128-multiples)
- z2 slice for the core's 49 windows stays in SBUF (baseline phase-Z2).
- The src-coord term is folded into the shipped x data on the host:
    x' = x_src + cx_src*u + cy_src*v with u,v = solve(W1a^T, -W1cx/-W1cy),
  so W1a^T x' = W1a^T x - W1c.c_src exactly (device sees 2 matmuls/tile):
    x-mm:  lhsT=x' tile, rhs=W1a'      (start)
    oh-mm: lhsT=oh tile, rhs=z2 window (stop)
- Tail per 8-tile group (2-bank psum tile): ACT relu -> bf16; GpSimd
  (Pool) folds each sign block in half; DVE reduces the halves, subtracts
  into the f32 outbuf.
- Octile window->core assignment (sorted by edge count) equalizes
  per-(core, local window) counts => ~3% shared-layout padding and
  balanced per-core edge loads.
"""

import numpy as np
import ml_dtypes

N, C, E = 50000, 128, 800000
N_PAD = 50176            # 392 * 128
N_CORES = 8
RNG = N_PAD // N_CORES   # 6272 dst nodes per core
W_PER_CORE = RNG // 128  # 49 dst windows per core
CH = 8192                # slots per streamed chunk (64 tiles)
GRP = 8                  # tiles per psum group (2 banks, 1024 slots)

_cache = {}


def _build(layout, p_pos):
    import concourse.bacc as bacc
    import concourse.mybir as mybir
    import concourse.tile as tile

    k_w = layout
    NTILES = sum(k_w)
    SLOTS = NTILES * 128
    w_of_tile = [w for w in range(W_PER_CORE) for _ in range(k_w[w])]
    bf16 = mybir.dt.bfloat16
    f32 = mybir.dt.float32
    fp8 = mybir.dt.float8e4

    nc = bacc.Bacc("TRN2", target_bir_lowering=False, debug=False,
                   num_devices=N_CORES)

    tokS = nc.dram_tensor("tokS", [C, RNG], bf16, kind="ExternalInput").ap()
    cooS4 = nc.dram_tensor("cooS4", [16, 13 * 128], bf16,
                           kind="ExternalInput").ap()
    w1ab = nc.dram_tensor("w1ab", [C, 2 * C], bf16, kind="ExternalInput").ap()
    w1c16 = nc.dram_tensor("w1c16", [16, 2 * 512], bf16,
                           kind="ExternalInput").ap()
    xsrcT = nc.dram_tensor("xsrcT", [128, SLOTS], bf16,
                           kind="ExternalInput").ap()
    ohd = nc.dram_tensor("oh", [128, SLOTS], fp8, kind="ExternalInput").ap()
    outd = nc.dram_tensor("out", [128, NTILES], f32, kind="ExternalOutput").ap()

    with tile.TileContext(nc) as tc:
        with (
            tc.tile_pool(name="wpool", bufs=1) as wpool,
            tc.tile_pool(name="ztok", bufs=1) as ztok,
            tc.tile_pool(name="czp", bufs=2) as czp,
            tc.tile_pool(name="z2p", bufs=1) as z2p,
            tc.tile_pool(name="xpool", bufs=5) as xpool,
            tc.tile_pool(name="ohp", bufs=5) as ohp,
            tc.tile_pool(name="gpsum", bufs=4, space="PSUM") as gpsum,
            tc.tile_pool(name="rpool", bufs=4) as rpool,
            tc.tile_pool(name="rfold", bufs=4) as rfold,
            tc.tile_pool(name="opool", bufs=6) as opool,
            tc.tile_pool(name="obuf", bufs=1) as obuf,
        ):
            w1ab_sb = wpool.tile([C, 2 * C], bf16)
            nc.sync.dma_start(out=w1ab_sb[:], in_=w1ab[:])
            w1c16_sb = wpool.tile([16, 2 * 512], bf16)
            nc.sync.dma_start(out=w1c16_sb[:], in_=w1c16[:])
            cooS4_sb = wpool.tile([16, 13 * 128], bf16)
            nc.sync.dma_start(out=cooS4_sb[:], in_=cooS4[:])
            outbuf = obuf.tile([128, NTILES], f32)

            # ---------------- phase Z2: per-core z2 slice in SBUF ------------
            tokS_sb = ztok.tile([C, RNG], bf16, tag="tokS")
            nc.sync.dma_start(out=tokS_sb[:], in_=tokS[:])
            z2sl = z2p.tile([128, W_PER_CORE, C], bf16)

            for qb in range(13):                 # 49 groups = 12*4 + 1
                q0 = qb * 4
                ngr = min(4, W_PER_CORE - q0)
                # token matmuls into quarters of the first half of a 2-bank
                # psum tile (shared with the edge-group pool); coord matmul
                # into the second half.
                bt = gpsum.tile([128, GRP * 128], f32, tag="gps")
                ps_t = bt[:, 0:512]
                ps_c = bt[:, 512:1024]
                for j in range(ngr):
                    cc = (q0 + j) * 128
                    nc.tensor.matmul(ps_t[:, j * 128:(j + 1) * 128],
                                     lhsT=tokS_sb[:, cc:cc + 128],
                                     rhs=w1ab_sb[:, C:2 * C],
                                     start=True, stop=True)
                nc.tensor.matmul(ps_c[:, 0:ngr * 128],
                                 lhsT=cooS4_sb[:, qb * 128:(qb + 1) * 128],
                                 rhs=w1c16_sb[:, 512:512 + ngr * 128],
                                 start=True, stop=True)
                cz = czp.tile([128, 4, C], bf16, tag="cz")
                nc.scalar.copy(out=cz[:, 0:ngr, :],
                               in_=ps_c[:, 0:ngr * 128].rearrange(
                                   "p (t c) -> p t c", c=C))
                nc.vector.scalar_tensor_tensor(
                    out=z2sl[:, q0:q0 + ngr, :],
                    in0=ps_t[:, 0:ngr * 128].rearrange("p (t c) -> p t c", c=C),
                    scalar=1.0, in1=cz[:, 0:ngr, :],
                    op0=mybir.AluOpType.mult, op1=mybir.AluOpType.add)

            # ---------------- edge stream ------------------------------------
            chunks = {}

            def issue_chunk(ci):
                s0 = ci * CH
                if s0 >= SLOTS or ci in chunks:
                    return
                cw = min(CH, SLOTS - s0)
                xt = xpool.tile([128, CH], bf16, tag="x")
                ot = ohp.tile([128, CH], fp8, tag="oh")
                for p0 in range(0, cw, 4096):
                    p1 = min(cw, p0 + 4096)
                    nc.sync.dma_start(out=xt[:, p0:p1],
                                      in_=xsrcT[:, s0 + p0:s0 + p1])
                    nc.sync.dma_start(out=ot[:, p0:p1],
                                      in_=ohd[:, s0 + p0:s0 + p1])
                chunks[ci] = (xt, ot)

            issue_chunk(0)
            issue_chunk(1)
            issue_chunk(2)
            issue_chunk(3)
            for t0 in range(0, NTILES, GRP):
                nj = min(GRP, NTILES - t0)
                ci = (t0 * 128) // CH
                issue_chunk(ci + 1)
                issue_chunk(ci + 2)
                issue_chunk(ci + 3)
                issue_chunk(ci + 4)
                xt, ot = chunks[ci]
                ps = gpsum.tile([128, GRP * 128], f32, tag="gps")
                for j in range(nj):
                    t = t0 + j
                    col = t * 128 - ci * CH
                    sl = ps[:, j * 128:(j + 1) * 128]
                    nc.tensor.matmul(sl, lhsT=xt[:, col:col + 128],
                                     rhs=w1ab_sb[:, 0:C],
                                     start=True, stop=False)
                    nc.tensor.matmul(sl, lhsT=ot[:, col:col + 128],
                                     rhs=z2sl[:, w_of_tile[t], :],
                                     start=False, stop=True)
                # free the chunk dict entry once past it (bufs recycle)
                if (t0 + nj) * 128 >= (ci + 1) * CH or t0 + nj >= NTILES:
                    chunks.pop(ci, None)
                r = rpool.tile([128, GRP, C], bf16, tag="r")
                rv = ps[:, 0:nj * 128].rearrange("p (t c) -> p t c", c=C)
                nc.scalar.activation(r[:, 0:nj, :], rv,
                                     mybir.ActivationFunctionType.Relu)
                assert p_pos % 2 == 0 and (C - p_pos) % 2 == 0, \
                    "gpsimd fold assumes even sign blocks"
                hp, hn = p_pos // 2, (C - p_pos) // 2
                # GpSimd (Pool engine, otherwise idle) folds both sign blocks
                # in half; DVE reduces the halves and subtracts.
                rf = rfold.tile([128, GRP, hp + hn], bf16, tag="rf")
                nc.gpsimd.tensor_tensor(
                    out=rf[:, 0:nj, 0:hp],
                    in0=r[:, 0:nj, 0:hp], in1=r[:, 0:nj, hp:p_pos],
                    op=mybir.AluOpType.add)
                nc.gpsimd.tensor_tensor(
                    out=rf[:, 0:nj, hp:hp + hn],
                    in0=r[:, 0:nj, p_pos:p_pos + hn],
                    in1=r[:, 0:nj, p_pos + hn:C],
                    op=mybir.AluOpType.add)
                o_pos = opool.tile([128, GRP], mybir.dt.float16, tag="op")
                o_neg = opool.tile([128, GRP], mybir.dt.float16, tag="on")
                with nc.allow_low_precision("fp16 store of channel sums"):
                    nc.vector.reduce_sum(o_pos[:, 0:nj],
                                         rf[:, 0:nj, 0:hp],
                                         axis=mybir.AxisListType.X)
                    nc.vector.reduce_sum(o_neg[:, 0:nj],
                                         rf[:, 0:nj, hp:hp + hn],
                                         axis=mybir.AxisListType.X)
                nc.vector.tensor_sub(outbuf[:, t0:t0 + nj],
                                     o_pos[:, 0:nj], o_neg[:, 0:nj])

            for o0 in range(0, NTILES, 256):
                o1 = min(NTILES, o0 + 256)
                nc.sync.dma_start(out=outd[:, o0:o1], in_=outbuf[:, o0:o1])

    nc.compile()
    return nc


def _prep_host(tokens, coords, edge_index, w1, b1, w2, b2):
    tokens = np.asarray(tokens, dtype=np.float32)[0]          # [N, C]
    coords = np.asarray(coords, dtype=np.float32)[0]          # [N, 2]
    ei = np.asarray(edge_index).astype(np.int64)              # [2, E]
    w1 = np.asarray(w1, dtype=np.float32)
    b1 = np.asarray(b1, dtype=np.float32)
    w2 = np.asarray(w2, dtype=np.float32)
    b2 = np.asarray(b2, dtype=np.float32)

    # fold |w2| into tables, sort channels by sign of w2
    w2v = w2[:, 0]
    order = np.argsort(w2v < 0, kind="stable")
    p_pos = int((w2v >= 0).sum())
    scale = np.abs(w2v[order])
    w1p = w1[:, order] * scale[None, :]
    b1p = b1[order] * scale
    W1a, W1b = w1p[:C], w1p[C:2 * C]
    W1cx, W1cy = w1p[2 * C], w1p[2 * C + 1]
    # coord-fold: u, v (from UNpermuted, UNscaled W1a/W1c) give
    # W1a^T (x + cx*u + cy*v) = W1a^T x - W1c.c_src exactly.
    u_fold = np.linalg.solve(w1[:C].astype(np.float64).T,
                             -w1[2 * C].astype(np.float64)).astype(np.float32)
    v_fold = np.linalg.solve(w1[:C].astype(np.float64).T,
                             -w1[2 * C + 1].astype(np.float64)).astype(np.float32)

    bf = ml_dtypes.bfloat16
    tokT_np = np.zeros((C, N_PAD), dtype=np.float32)
    tokT_np[:, :N] = tokens.T
    tokT_np = tokT_np.astype(bf)
    cooP = np.zeros((4, N_PAD), dtype=np.float32)
    cooP[0, :N] = coords[:, 0]
    cooP[1, :N] = coords[:, 1]
    cooP[2, :] = 1.0
    w1ab_np = np.concatenate([W1a, W1b], axis=1).astype(bf)   # [C, 256]
    # w1c16: [16, 2*512] block-diagonal; z1 half planes (-W1cx, -W1cy, 0, 0)
    # (rows 0:2 of block 0 are the per-edge src-coord rhs), z2 half
    # (+W1cx, +W1cy, b1, 0) for the phase-Z2 coord matmul.
    w1c16_np = np.zeros((16, 2 * 512), dtype=np.float32)
    planes_z1 = [-W1cx, -W1cy, np.zeros(C, np.float32), np.zeros(C, np.float32)]
    planes_z2 = [W1cx, W1cy, b1p, np.zeros(C, np.float32)]
    for j in range(4):
        for k in range(4):
            w1c16_np[4 * j + k, j * 128:(j + 1) * 128] = planes_z1[k]
            w1c16_np[4 * j + k, 512 + j * 128:512 + (j + 1) * 128] = planes_z2[k]
    w1c16_np = w1c16_np.astype(bf)

    src, dst = ei[0], ei[1]
    # octile window->core assignment: sort the 392 global 128-node dst
    # windows by edge count; local slot l gets windows rank 8l..8l+7, one
    # per core (largest to the least-loaded core). This equalizes the
    # per-(core, local-window) counts, minimizing shared-layout padding.
    NWIN = N_PAD // 128
    gw_all = dst // 128
    gcnt = np.bincount(gw_all, minlength=NWIN)
    order_w = np.argsort(-gcnt, kind="stable")
    core_of_win = np.empty(NWIN, dtype=np.int64)
    local_of_win = np.empty(NWIN, dtype=np.int64)
    loads = np.zeros(N_CORES, dtype=np.int64)
    for l in range(W_PER_CORE):
        for wgi in order_w[N_CORES * l:N_CORES * (l + 1)]:
            c = int(np.argmin(loads))
            loads[c] += gcnt[wgi]
            core_of_win[wgi] = c
            local_of_win[wgi] = l
    win_of_local = np.empty((N_CORES, W_PER_CORE), dtype=np.int64)
    win_of_local[core_of_win, local_of_win] = np.arange(NWIN)

    core = core_of_win[gw_all]
    w_all = local_of_win[gw_all]
    dloc = dst % 128

    cnt = np.zeros((N_CORES, W_PER_CORE), dtype=np.int64)
    np.add.at(cnt, (core, w_all), 1)
    k_w = tuple(int(x) for x in -(-cnt.max(axis=0) // 128))
    NTILES = int(sum(k_w))
    SLOTS = NTILES * 128
    wbase = np.concatenate([[0], np.cumsum(k_w)[:-1]]) * 128

    in_maps = []
    pos_maps = []
    eidx = np.arange(E, dtype=np.int64)
    for c in range(N_CORES):
        m = core == c
        o = np.argsort(w_all[m], kind="stable")
        gw = w_all[m][o]
        gs, gd, gp = src[m][o], dloc[m][o], eidx[m][o]
        kcnt = np.bincount(gw, minlength=W_PER_CORE)
        rank = np.arange(len(gw)) - np.repeat(
            np.concatenate([[0], kcnt.cumsum()[:-1]]), kcnt)
        slot = wbase[gw] + rank

        xcols = (tokens[gs]
                 + coords[gs, 0:1] * u_fold[None, :]
                 + coords[gs, 1:2] * v_fold[None, :])      # [m, C] f32
        xsrcT = np.zeros((C, SLOTS), dtype=bf)
        xsrcT[:, slot] = xcols.T.astype(bf)
        oh = np.zeros((128, SLOTS), dtype=ml_dtypes.float8_e4m3)
        oh[gd, slot] = 1.0
        pm = np.full(SLOTS, -1, dtype=np.int64)
        pm[slot] = gp

        # node columns of this core's 49 windows, in local-window order
        nidx = (win_of_local[c][:, None] * 128
                + np.arange(128)[None, :]).reshape(-1)
        cooS = cooP[:, nidx]
        cooS_pad = np.zeros((4, 13 * 4 * 128), dtype=np.float32)
        cooS_pad[:, :RNG] = cooS
        cooS4 = np.ascontiguousarray(
            cooS_pad.reshape(4, 13, 4, 128).transpose(2, 0, 1, 3)
            .reshape(16, 13 * 128)).astype(bf)
        in_maps.append({
            "tokS": np.ascontiguousarray(tokT_np[:, nidx]),
            "cooS4": cooS4,
            "w1ab": w1ab_np, "w1c16": w1c16_np,
            "xsrcT": xsrcT, "oh": oh,
        })
        pos_maps.append(pm)

    return k_w, p_pos, in_maps, pos_maps, float(b2[0])


def _spot_check(out, tokens, coords, edge_index, w1, b1, w2, b2, n=5000):
    # numpy reference on a random edge subsample; catches silent device
    # corruption
    rng = np.random.default_rng(12345)
    ei = np.asarray(edge_index).astype(np.int64)
    idx = rng.integers(0, E, n)
    src, dst = ei[0, idx], ei[1, idx]
    tok = np.asarray(tokens, dtype=np.float32)[0]
    coo = np.asarray(coords, dtype=np.float32)[0]
    feat = np.concatenate([tok[src], tok[dst], coo[dst] - coo[src]], axis=1)
    h = np.maximum(feat @ np.asarray(w1, np.float32)
                   + np.asarray(b1, np.float32), 0.0)
    exp = h @ np.asarray(w2, np.float32)[:, 0] + np.asarray(b2, np.float32)[0]
    return np.abs(out[idx] - exp).max()


def kernel(tokens, coords, edge_index, w1, b1, w2, b2):
    from concourse.bass_utils import run_bass_kernel_spmd

    k_w, p_pos, in_maps, pos_maps, b2v = _prep_host(
        tokens, coords, edge_index, w1, b1, w2, b2)

    key = (k_w, p_pos)
    if key not in _cache:
        _cache[key] = _build(k_w, p_pos)
    nc = _cache[key]

    last_err = None
    out = None
    for _attempt in range(4):
        try:
            res = run_bass_kernel_spmd(nc, in_maps, list(range(N_CORES)))
        except Exception as e:  # transient NRT exec-unit errors observed
            last_err = e
            import time as _time
            _time.sleep(20)
            continue
        out = np.empty(E, dtype=np.float32)
        for c in range(N_CORES):
            o = res.results[c]["out"]              # [128, NTILES]
            flat = o.T.reshape(-1)                 # slot = t*128 + p
            pm = pos_maps[c]
            valid = pm >= 0
            out[pm[valid]] = flat[valid]
        out += b2v
        err = _spot_check(out, tokens, coords, edge_index, w1, b1, w2, b2)
        if err < 0.05:                             # bf16 noise ~0.013
            break
    if out is None:
        raise last_err
    return out.reshape(1, E, 1)


# revision 3
# speedup vs baseline: 1.0509x; 1.0499x over previous
"""Trainium2 Bass kernel for nn_AffinityHead (no-gather, coord-folded).

Math: out[e] = w2 . relu(W1a.x_src + W1b.x_dst + W1c.(c_dst - c_src) + b1) + b2
Per-node z2 table (dst side): z2[n] = x_n@W1b + W1c.c_n + b1, |w2| folded,
channels sign-sorted. Per edge: out[e] = +/-sum relu(x_src@W1a + z2[dst]
- W1c.c_src) + b2.

Structure (per core; edges sharded by dst range, 6272 nodes / 49 windows):
- NO SWDGE gather, NO z1 table. The host ships, per core, slot-ordered:
    xsrcT [128, SLOTS] bf16 : pre-gathered tokens[src].T per edge slot
    oh    [128, SLOTS] fp8  : one-hot of dst-within-window per slot
  (slots = edges sorted by dst window, padded per window to 128-multiples)
- z2 slice for the core's 49 windows stays in SBUF (baseline phase-Z2).
- The src-coord term is folded into the shipped x data on the host:
    x' = x_src + cx_src*u + cy_src*v with u,v = solve(W1a^T, -W1cx/-W1cy),
  so W1a^T x' = W1a^T x - W1c.c_src exactly (device sees 2 matmuls/tile):
    x-mm:  lhsT=x' tile, rhs=W1a'      (start)
    oh-mm: lhsT=oh tile, rhs=z2 window (stop)
- Tail per 8-tile group (2-bank psum tile): ACT relu -> bf16; GpSimd
  (Pool) folds each sign block in half; DVE reduces the halves, subtracts
  into the f32 outbuf.
- Octile window->core assignment (sorted by edge count) equalizes
  per-(core, local window) counts => ~3% shared-layout padding and
  balanced per-core edge loads.
"""

import numpy as np
import ml_dtypes

N, C, E = 50000, 128, 800000
N_PAD = 50176            # 392 * 128
N_CORES = 8
RNG = N_PAD // N_CORES   # 6272 dst nodes per core
W_PER_CORE = RNG // 128  # 49 dst windows per core
CH = 8192                # slots per streamed chunk (64 tiles)
GRP = 8                  # tiles per psum group (2 banks, 1024 slots)

_cache = {}


def _build(layout, p_pos):
    import concourse.bacc as bacc
    import concourse.mybir as mybir
    import concourse.tile as tile

    k_w = layout
    NTILES = sum(k_w)
    SLOTS = NTILES * 128
    w_of_tile = [w for w in range(W_PER_CORE) for _ in range(k_w[w])]
    bf16 = mybir.dt.bfloat16
    f32 = mybir.dt.float32
    fp8 = mybir.dt.float8e4

    nc = bacc.Bacc("TRN2", target_bir_lowering=False, debug=False,
                   num_devices=N_CORES)

    tokS = nc.dram_tensor("tokS", [C, RNG], bf16, kind="ExternalInput").ap()
    cooS4 = nc.dram_tensor("cooS4", [16, 13 * 128], bf16,
                           kind="ExternalInput").ap()
    w1ab = nc.dram_tensor("w1ab", [C, 2 * C], bf16, kind="ExternalInput").ap()
    w1c16 = nc.dram_tensor("w1c16", [16, 2 * 512], bf16,
                           kind="ExternalInput").ap()
    xsrcT = nc.dram_tensor("xsrcT", [128, SLOTS], bf16,
                           kind="ExternalInput").ap()
    ohd = nc.dram_tensor("oh", [128, SLOTS], fp8, kind="ExternalInput").ap()
    outd = nc.dram_tensor("out", [128, NTILES], f32, kind="ExternalOutput").ap()

    with tile.TileContext(nc) as tc:
        with (
            tc.tile_pool(name="wpool", bufs=1) as wpool,
            tc.tile_pool(name="ztok", bufs=1) as ztok,
            tc.tile_pool(name="czp", bufs=2) as czp,
            tc.tile_pool(name="z2p", bufs=1) as z2p,
            tc.tile_pool(name="xpool", bufs=5) as xpool,
            tc.tile_pool(name="ohp", bufs=5) as ohp,
            tc.tile_pool(name="gpsum", bufs=4, space="PSUM") as gpsum,
            tc.tile_pool(name="rpool", bufs=4) as rpool,
            tc.tile_pool(name="rfold", bufs=4) as rfold,
            tc.tile_pool(name="opool", bufs=6) as opool,
            tc.tile_pool(name="obuf", bufs=1) as obuf,
        ):
            w1ab_sb = wpool.tile([C, 2 * C], bf16)
            nc.sync.dma_start(out=w1ab_sb[:], in_=w1ab[:])
            w1c16_sb = wpool.tile([16, 2 * 512], bf16)
            nc.sync.dma_start(out=w1c16_sb[:], in_=w1c16[:])
            cooS4_sb = wpool.tile([16, 13 * 128], bf16)
            nc.sync.dma_start(out=cooS4_sb[:], in_=cooS4[:])
            outbuf = obuf.tile([128, NTILES], f32)

            # ---------------- phase Z2: per-core z2 slice in SBUF ------------
            tokS_sb = ztok.tile([C, RNG], bf16, tag="tokS")
            nc.sync.dma_start(out=tokS_sb[:], in_=tokS[:])
            z2sl = z2p.tile([128, W_PER_CORE, C], bf16)

            for qb in range(13):                 # 49 groups = 12*4 + 1
                q0 = qb * 4
                ngr = min(4, W_PER_CORE - q0)
                # token matmuls into quarters of the first half of a 2-bank
                # psum tile (shared with the edge-group pool); coord matmul
                # into the second half.
                bt = gpsum.tile([128, GRP * 128], f32, tag="gps")
                ps_t = bt[:, 0:512]
                ps_c = bt[:, 512:1024]
                for j in range(ngr):
                    cc = (q0 + j) * 128
                    nc.tensor.matmul(ps_t[:, j * 128:(j + 1) * 128],
                                     lhsT=tokS_sb[:, cc:cc + 128],
                                     rhs=w1ab_sb[:, C:2 * C],
                                     start=True, stop=True)
                nc.tensor.matmul(ps_c[:, 0:ngr * 128],
                                 lhsT=cooS4_sb[:, qb * 128:(qb + 1) * 128],
                                 rhs=w1c16_sb[:, 512:512 + ngr * 128],
                                 start=True, stop=True)
                cz = czp.tile([128, 4, C], bf16, tag="cz")
                nc.scalar.copy(out=cz[:, 0:ngr, :],
                               in_=ps_c[:, 0:ngr * 128].rearrange(
                                   "p (t c) -> p t c", c=C))
                nc.vector.scalar_tensor_tensor(
                    out=z2sl[:, q0:q0 + ngr, :],
                    in0=ps_t[:, 0:ngr * 128].rearrange("p (t c) -> p t c", c=C),
                    scalar=1.0, in1=cz[:, 0:ngr, :],
                    op0=mybir.AluOpType.mult, op1=mybir.AluOpType.add)

            # ---------------- edge stream ------------------------------------
            chunks = {}

            def issue_chunk(ci):
                s0 = ci * CH
                if s0 >= SLOTS or ci in chunks:
                    return
                cw = min(CH, SLOTS - s0)
                xt = xpool.tile([128, CH], bf16, tag="x")
                ot = ohp.tile([128, CH], fp8, tag="oh")
                for p0 in range(0, cw, 4096):
                    p1 = min(cw, p0 + 4096)
                    nc.sync.dma_start(out=xt[:, p0:p1],
                                      in_=xsrcT[:, s0 + p0:s0 + p1])
                    nc.sync.dma_start(out=ot[:, p0:p1],
                                      in_=ohd[:, s0 + p0:s0 + p1])
                chunks[ci] = (xt, ot)

            issue_chunk(0)
            issue_chunk(1)
            issue_chunk(2)
            issue_chunk(3)
            for t0 in range(0, NTILES, GRP):
                nj = min(GRP, NTILES - t0)
                ci = (t0 * 128) // CH
                issue_chunk(ci + 1)
                issue_chunk(ci + 2)
                issue_chunk(ci + 3)
                issue_chunk(ci + 4)
                xt, ot = chunks[ci]
                ps = gpsum.tile([128, GRP * 128], f32, tag="gps")
                for j in range(nj):
                    t = t0 + j
                    col = t * 128 - ci * CH
                    sl = ps[:, j * 128:(j + 1) * 128]
                    nc.tensor.matmul(sl, lhsT=xt[:, col:col + 128],
                                     rhs=w1ab_sb[:, 0:C],
                                     start=True, stop=False)
                    nc.tensor.matmul(sl, lhsT=ot[:, col:col + 128],
                                     rhs=z2sl[:, w_of_tile[t], :],
                                     start=False, stop=True)
                # free the chunk dict entry once past it (bufs recycle)
                if (t0 + nj) * 128 >= (ci + 1) * CH or t0 + nj >= NTILES:
                    chunks.pop(ci, None)
                r = rpool.tile([128, GRP, C], bf16, tag="r")
                rv = ps[:, 0:nj * 128].rearrange("p (t c) -> p t c", c=C)
                nc.scalar.activation(r[:, 0:nj, :], rv,
                                     mybir.ActivationFunctionType.Relu)
                o_pos = opool.tile([128, GRP], mybir.dt.float16, tag="op")
                o_neg = opool.tile([128, GRP], mybir.dt.float16, tag="on")
                if p_pos % 2 == 0 and (C - p_pos) % 2 == 0 and 0 < p_pos < C:
                    hp, hn = p_pos // 2, (C - p_pos) // 2
                    # GpSimd (Pool engine, otherwise idle) folds both sign
                    # blocks in half; DVE reduces the halves and subtracts.
                    rf = rfold.tile([128, GRP, hp + hn], bf16, tag="rf")
                    nc.gpsimd.tensor_tensor(
                        out=rf[:, 0:nj, 0:hp],
                        in0=r[:, 0:nj, 0:hp], in1=r[:, 0:nj, hp:p_pos],
                        op=mybir.AluOpType.add)
                    nc.gpsimd.tensor_tensor(
                        out=rf[:, 0:nj, hp:hp + hn],
                        in0=r[:, 0:nj, p_pos:p_pos + hn],
                        in1=r[:, 0:nj, p_pos + hn:C],
                        op=mybir.AluOpType.add)
                    with nc.allow_low_precision("fp16 store of channel sums"):
                        nc.vector.reduce_sum(o_pos[:, 0:nj],
                                             rf[:, 0:nj, 0:hp],
                                             axis=mybir.AxisListType.X)
                        nc.vector.reduce_sum(o_neg[:, 0:nj],
                                             rf[:, 0:nj, hp:hp + hn],
                                             axis=mybir.AxisListType.X)
                else:
                    # fallback for odd/degenerate sign splits: plain DVE
                    # reduces over the raw blocks
                    with nc.allow_low_precision("fp16 store of channel sums"):
                        if p_pos > 0:
                            nc.vector.reduce_sum(o_pos[:, 0:nj],
                                                 r[:, 0:nj, 0:p_pos],
                                                 axis=mybir.AxisListType.X)
                        else:
                            nc.vector.memset(o_pos[:, 0:nj], 0.0)
                        if p_pos < C:
                            nc.vector.reduce_sum(o_neg[:, 0:nj],
                                                 r[:, 0:nj, p_pos:C],
                                                 axis=mybir.AxisListType.X)
                        else:
                            nc.vector.memset(o_neg[:, 0:nj], 0.0)
                nc.vector.tensor_sub(outbuf[:, t0:t0 + nj],
                                     o_pos[:, 0:nj], o_neg[:, 0:nj])

            for o0 in range(0, NTILES, 256):
                o1 = min(NTILES, o0 + 256)
                nc.sync.dma_start(out=outd[:, o0:o1], in_=outbuf[:, o0:o1])

    nc.compile()
    return nc


def _prep_host(tokens, coords, edge_index, w1, b1, w2, b2):
    tokens = np.asarray(tokens, dtype=np.float32)[0]          # [N, C]
    coords = np.asarray(coords, dtype=np.float32)[0]          # [N, 2]
    ei = np.asarray(edge_index).astype(np.int64)              # [2, E]
    w1 = np.asarray(w1, dtype=np.float32)
    b1 = np.asarray(b1, dtype=np.float32)
    w2 = np.asarray(w2, dtype=np.float32)
    b2 = np.asarray(b2, dtype=np.float32)

    # fold |w2| into tables, sort channels by sign of w2
    w2v = w2[:, 0]
    order = np.argsort(w2v < 0, kind="stable")
    p_pos = int((w2v >= 0).sum())
    scale = np.abs(w2v[order])
    w1p = w1[:, order] * scale[None, :]
    b1p = b1[order] * scale
    W1a, W1b = w1p[:C], w1p[C:2 * C]
    W1cx, W1cy = w1p[2 * C], w1p[2 * C + 1]
    # coord-fold: u, v (from UNpermuted, UNscaled W1a/W1c) give
    # W1a^T (x + cx*u + cy*v) = W1a^T x - W1c.c_src exactly.
    u_fold = np.linalg.solve(w1[:C].astype(np.float64).T,
                             -w1[2 * C].astype(np.float64)).astype(np.float32)
    v_fold = np.linalg.solve(w1[:C].astype(np.float64).T,
                             -w1[2 * C + 1].astype(np.float64)).astype(np.float32)

    bf = ml_dtypes.bfloat16
    tokT_np = np.zeros((C, N_PAD), dtype=np.float32)
    tokT_np[:, :N] = tokens.T
    tokT_np = tokT_np.astype(bf)
    cooP = np.zeros((4, N_PAD), dtype=np.float32)
    cooP[0, :N] = coords[:, 0]
    cooP[1, :N] = coords[:, 1]
    cooP[2, :] = 1.0
    w1ab_np = np.concatenate([W1a, W1b], axis=1).astype(bf)   # [C, 256]
    # w1c16: [16, 2*512] block-diagonal; z1 half planes (-W1cx, -W1cy, 0, 0)
    # (rows 0:2 of block 0 are the per-edge src-coord rhs), z2 half
    # (+W1cx, +W1cy, b1, 0) for the phase-Z2 coord matmul.
    w1c16_np = np.zeros((16, 2 * 512), dtype=np.float32)
    planes_z1 = [-W1cx, -W1cy, np.zeros(C, np.float32), np.zeros(C, np.float32)]
    planes_z2 = [W1cx, W1cy, b1p, np.zeros(C, np.float32)]
    for j in range(4):
        for k in range(4):
            w1c16_np[4 * j + k, j * 128:(j + 1) * 128] = planes_z1[k]
            w1c16_np[4 * j + k, 512 + j * 128:512 + (j + 1) * 128] = planes_z2[k]
    w1c16_np = w1c16_np.astype(bf)

    src, dst = ei[0], ei[1]
    # octile window->core assignment: sort the 392 global 128-node dst
    # windows by edge count; local slot l gets windows rank 8l..8l+7, one
    # per core (largest to the least-loaded core). This equalizes the
    # per-(core, local-window) counts, minimizing shared-layout padding.
    NWIN = N_PAD // 128
    gw_all = dst // 128
    gcnt = np.bincount(gw_all, minlength=NWIN)
    order_w = np.argsort(-gcnt, kind="stable")
    core_of_win = np.empty(NWIN, dtype=np.int64)
    local_of_win = np.empty(NWIN, dtype=np.int64)
    loads = np.zeros(N_CORES, dtype=np.int64)
    for l in range(W_PER_CORE):
        for wgi in order_w[N_CORES * l:N_CORES * (l + 1)]:
            c = int(np.argmin(loads))
            loads[c] += gcnt[wgi]
            core_of_win[wgi] = c
            local_of_win[wgi] = l
    win_of_local = np.empty((N_CORES, W_PER_CORE), dtype=np.int64)
    win_of_local[core_of_win, local_of_win] = np.arange(NWIN)

    core = core_of_win[gw_all]
    w_all = local_of_win[gw_all]
    dloc = dst % 128

    cnt = np.zeros((N_CORES, W_PER_CORE), dtype=np.int64)
    np.add.at(cnt, (core, w_all), 1)
    k_w = tuple(int(x) for x in -(-cnt.max(axis=0) // 128))
    NTILES = int(sum(k_w))
    SLOTS = NTILES * 128
    wbase = np.concatenate([[0], np.cumsum(k_w)[:-1]]) * 128

    in_maps = []
    pos_maps = []
    eidx = np.arange(E, dtype=np.int64)
    for c in range(N_CORES):
        m = core == c
        o = np.argsort(w_all[m], kind="stable")
        gw = w_all[m][o]
        gs, gd, gp = src[m][o], dloc[m][o], eidx[m][o]
        kcnt = np.bincount(gw, minlength=W_PER_CORE)
        rank = np.arange(len(gw)) - np.repeat(
            np.concatenate([[0], kcnt.cumsum()[:-1]]), kcnt)
        slot = wbase[gw] + rank

        xcols = (tokens[gs]
                 + coords[gs, 0:1] * u_fold[None, :]
                 + coords[gs, 1:2] * v_fold[None, :])      # [m, C] f32
        xsrcT = np.zeros((C, SLOTS), dtype=bf)
        xsrcT[:, slot] = xcols.T.astype(bf)
        oh = np.zeros((128, SLOTS), dtype=ml_dtypes.float8_e4m3)
        oh[gd, slot] = 1.0
        pm = np.full(SLOTS, -1, dtype=np.int64)
        pm[slot] = gp

        # node columns of this core's 49 windows, in local-window order
        nidx = (win_of_local[c][:, None] * 128
                + np.arange(128)[None, :]).reshape(-1)
        cooS = cooP[:, nidx]
        cooS_pad = np.zeros((4, 13 * 4 * 128), dtype=np.float32)
        cooS_pad[:, :RNG] = cooS
        cooS4 = np.ascontiguousarray(
            cooS_pad.reshape(4, 13, 4, 128).transpose(2, 0, 1, 3)
            .reshape(16, 13 * 128)).astype(bf)
        in_maps.append({
            "tokS": np.ascontiguousarray(tokT_np[:, nidx]),
            "cooS4": cooS4,
            "w1ab": w1ab_np, "w1c16": w1c16_np,
            "xsrcT": xsrcT, "oh": oh,
        })
        pos_maps.append(pm)

    return k_w, p_pos, in_maps, pos_maps, float(b2[0])


def _spot_check(out, tokens, coords, edge_index, w1, b1, w2, b2, n=5000):
    # numpy reference on a random edge subsample; catches silent device
    # corruption
    rng = np.random.default_rng(12345)
    ei = np.asarray(edge_index).astype(np.int64)
    idx = rng.integers(0, E, n)
    src, dst = ei[0, idx], ei[1, idx]
    tok = np.asarray(tokens, dtype=np.float32)[0]
    coo = np.asarray(coords, dtype=np.float32)[0]
    feat = np.concatenate([tok[src], tok[dst], coo[dst] - coo[src]], axis=1)
    h = np.maximum(feat @ np.asarray(w1, np.float32)
                   + np.asarray(b1, np.float32), 0.0)
    exp = h @ np.asarray(w2, np.float32)[:, 0] + np.asarray(b2, np.float32)[0]
    return np.abs(out[idx] - exp).max()


def kernel(tokens, coords, edge_index, w1, b1, w2, b2):
    from concourse.bass_utils import run_bass_kernel_spmd

    k_w, p_pos, in_maps, pos_maps, b2v = _prep_host(
        tokens, coords, edge_index, w1, b1, w2, b2)

    key = (k_w, p_pos)
    if key not in _cache:
        _cache[key] = _build(k_w, p_pos)
    nc = _cache[key]

    last_err = None
    out = None
    for _attempt in range(4):
        try:
            res = run_bass_kernel_spmd(nc, in_maps, list(range(N_CORES)))
        except Exception as e:  # transient NRT exec-unit errors observed
            last_err = e
            import time as _time
            _time.sleep(20)
            continue
        out = np.empty(E, dtype=np.float32)
        for c in range(N_CORES):
            o = res.results[c]["out"]              # [128, NTILES]
            flat = o.T.reshape(-1)                 # slot = t*128 + p
            pm = pos_maps[c]
            valid = pm >= 0
            out[pm[valid]] = flat[valid]
        out += b2v
        err = _spot_check(out, tokens, coords, edge_index, w1, b1, w2, b2)
        if err < 0.05:                             # bf16 noise ~0.013
            break
    if out is None:
        raise last_err
    return out.reshape(1, E, 1)
